# revision 3
# baseline (speedup 1.0000x reference)
"""Trainium2 Bass kernel for AdaConv2d (instance-norm + per-sample dynamic
depthwise 3x3 conv + per-channel scale/bias + shared dense 3x3 conv, reflect
padding everywhere).

Data-parallel over batch: 8 samples -> 8 NeuronCores, one sample per core.

Math (per sample, per channel c):
    xn   = (x - mu_c) * rsqrt(var_c + eps)
    mid  = wp_c * depthwise3x3(reflect_pad(xn); ws_c) + b_c
         = v_c + b_c    with v = a*dw(x) - a*mu*s9,  a = wp*rsqrt(var+eps)
    out  = dense3x3(reflect_pad(mid); conv_w) + conv_b
         = dense3x3(reflect_pad(v); conv_w) + S@b + conv_b
      where S[co,ci] = sum_taps conv_w  (reflect padding makes the per-channel
      constant b contribute exactly S@b at every output pixel).  S@b + conv_b
      is exact host-side preprocessing of the weights/bias (input `ab`).

The v split is what makes fp8 viable: v has std ~0.0075 (vs mid's ~0.05
dominated by the per-channel constant b), so quantizing v*2^9 to fp8e4m3
keeps the dense-conv error ~1e-2 relative (gate is 2e-2).  The dense conv
runs as fp8 DoubleRow matmuls: each matmul contracts 256 = 2 channel tiles
(the pair dim of the moving AP strides between two v images that live in
one SBUF tile).  On HW the PE streams 1 moving column/cycle regardless of
mode, so DR's win is halving the accumulation passes (18 per co tile).

Depthwise runs on the PE as diagonal-fp8-weight matmuls, DoubleRow-pairing
two taps per matmul.  Tiles are produced in order (2,3,0,1); the first two
keep all 5 tap-blocks (4 pairs + center) on the PE; the last two drop the
center tap, which rides a GpSimd tensor_scalar pass t2 = (a*w_c)*x + tb
(their instance-norm stats are ready early enough for that by then).  MMs
are chunk-outer (each 8-row chunk finishes all its tap blocks back to back)
so the per-chunk psum->fp16 staging copies on the vector engine recycle the
8 psum banks at chunk cadence, which is what lets the next tile / the dense
conv start without PE gaps.  The per-tile affine v = a*acc (+ t2|+ tb) is
emitted with ~1-tile lag in the DVE FIFO so it never head-of-line blocks
the staging copies.  The scalar engine runs the two stats passes per tile
(Square/Identity with accum) + sqrt; small ops ride DVE; the stats smalls
are split pre-/post-sqrt and placed so no engine FIFO stalls another.

Head: the first matmul's dependencies (tile-2 diag weights + x rows 0-17)
ride the otherwise-idle tensor-engine DMA ring, with the rest of tile 2
split across the sync/gpsimd rings, so the PE starts ~3us earlier.  Tail:
the last dense co tile runs chunk-outer (18 accum MMs per chunk, then
immediate eviction + output DMA) so only one chunk's eviction remains
after the final matmul.
"""

import os
import sys
import types

import numpy as np
import ml_dtypes

B, C, H, W = 8, 512, 64, 64
KS = 3
EPS = 1e-5
N_CORES = 8
P = 128
CT = C // P            # 4 channel tiles
PADH = H + 2           # 66
PADW = W + 2           # 66
IMGC = 72              # stored row stride (16B-aligned images: 66*72 = 4752)
IMG = PADH * IMGC      # 4752
HW = H * W             # 4096
NCHUNK = 8             # 8-row output chunks -> one psum bank each
TILE_ORDER = (2, 3, 0, 1)  # depthwise production order (dense eats 2,3 first)
CI_PAIRS = ((0, 1), (2, 3))
PAIR_ORDER = (1, 0)    # dense contraction: pair (2,3) first (ready early)
K_SC = 9               # v scale 2^9
M_SC = 9               # dense weight scale 2^9
S_SC = 9               # depthwise diag weight scale 2^9
OUT_SCALE = float(2.0 ** (-(K_SC + M_SC)))
# depthwise tap blocks: 4 DoubleRow pairs + 1 single (center)
DW_BLOCKS = (((0, 0), (0, 2)), ((1, 0), (1, 2)), ((2, 0), (2, 2)),
             ((0, 1), (2, 1)), ((1, 1),))
GPS_TILES = (0, 1)     # tiles whose center tap rides GpSimd (produced last)

E4 = ml_dtypes.float8_e4m3


def _install_ntff_hook():
    """Register the NTFF profiling hook that concourse expects under axon
    (missing antenv.axon_hooks module in this image)."""
    if "antenv.axon_hooks" in sys.modules:
        return
    try:
        mod = types.ModuleType("antenv.axon_hooks")
        holder = [None]
        mod.set_axon_ntff_profile_hook = lambda h: holder.__setitem__(0, h)
        mod.get_axon_ntff_profile_hook = lambda: holder[0]
        sys.modules["antenv.axon_hooks"] = mod
        from trn_agent_boot.trn_boot import _ntff_profile_via_ctypes

        hook = _ntff_profile_via_ctypes("/opt/axon/libaxon_pjrt.so")
        mod.set_axon_ntff_profile_hook(hook)
    except Exception:
        sys.modules.pop("antenv.axon_hooks", None)


_TRACE = os.environ.get("BASS_KERNEL_TRACE") == "1"
if _TRACE:
    _install_ntff_hook()

import concourse.tile as tile
from concourse import bacc, mybir
import concourse.bass_utils as bass_utils
from concourse.bass_utils import run_bass_kernel_spmd
from concourse.ap import AP

if _TRACE:
    bass_utils.upload_artifacts = lambda d: d

LAST_EXEC_NS = None
_CACHE = {}


def _taps():
    for tap in range(KS * KS):
        yield tap, tap // KS, tap % KS


def _reflect_borders(nc, img3):
    """Fill the 1-wide reflect border of a [128, 66, 72] image whose
    interior [1:65, 1:65] is already populated (cols first, then full rows
    so the corners come out as reflect-of-reflect, matching np.pad)."""
    nc.vector.tensor_copy(img3[:, 1:H + 1, 0:1], img3[:, 1:H + 1, 2:3])
    nc.vector.tensor_copy(img3[:, 1:H + 1, PADW - 1:PADW],
                          img3[:, 1:H + 1, PADW - 3:PADW - 2])
    nc.vector.tensor_copy(img3[:, 0:1, 0:PADW], img3[:, 2:3, 0:PADW])
    nc.vector.tensor_copy(img3[:, PADH - 1:PADH, 0:PADW],
                          img3[:, PADH - 3:PADH - 2, 0:PADW])


def _dedup_ldweights(nc):
    """Drop InstLdweights whose weights AP is identical to the previous
    weight load on the PE stream (bacc splits every matmul into LDW+MM;
    with one weight block reused across 8 chunk matmuls, 7 of 8 loads are
    redundant and serialize with the matmuls).  LDWs carrying semaphore
    waits/updates are kept."""
    n_removed = 0
    for f in nc.m.functions:
        for bb in f.blocks:
            insts = bb.instructions
            keep = []
            last_key = None
            for inst in insts:
                tn = type(inst).__name__
                if tn == "InstLdweights":
                    si = inst.sync_info
                    has_sync = si is not None and (
                        len(si.on_wait) > 0 or len(si.on_update) > 0
                    )
                    key = repr(inst.ins[0])
                    if key == last_key and not has_sync:
                        n_removed += 1
                        continue
                    last_key = key
                elif tn == "InstMatmult":
                    if getattr(inst, "is_transpose", False):
                        last_key = None
                keep.append(inst)
            if len(keep) != len(insts):
                bb.instructions = keep
    return n_removed


def _build():
    nc = bacc.Bacc("TRN2", target_bir_lowering=False, debug=False,
                   num_devices=N_CORES)
    f32 = mybir.dt.float32
    f16 = mybir.dt.float16
    f8 = mybir.dt.float8e4
    DR = mybir.MatmulPerfMode.DoubleRow

    x8_in = nc.dram_tensor("x8", [C, IMG], f8, kind="ExternalInput").ap()
    wt_in = nc.dram_tensor("wt", [P, 72 * 256], f8, kind="ExternalInput").ap()
    dg_in = nc.dram_tensor("dg", [P, 4 * 1152], f8, kind="ExternalInput").ap()
    prm_in = nc.dram_tensor("prm", [P, CT * 11], f32,
                            kind="ExternalInput").ap()
    ab_in = nc.dram_tensor("ab", [P, CT], f32, kind="ExternalInput").ap()
    out_ext = nc.dram_tensor("out", [C, HW], f16, kind="ExternalOutput").ap()

    with tile.TileContext(nc) as tc:
        with (
            tc.tile_pool(name="wpool", bufs=1) as wpool,
            tc.tile_pool(name="xpool", bufs=2) as xpool,
            tc.tile_pool(name="vpool", bufs=1) as vpool,
            tc.tile_pool(name="accpool", bufs=2) as accpool,
            tc.tile_pool(name="t2pool", bufs=2) as t2pool,
            tc.tile_pool(name="ypool", bufs=2) as ypool,
            tc.tile_pool(name="smpool", bufs=8) as smpool,
            tc.tile_pool(name="prmpool", bufs=4) as prmpool,
            tc.tile_pool(name="opool", bufs=8) as opool,
            tc.tile_pool(name="psum", bufs=8, space="PSUM") as psum,
        ):
            # ---- input DMAs: the first matmul (tile 2 chunk 0) needs only
            # the tile-2 diag weights + x rows 0-17; both ride the idle
            # tensor-engine ring so the PE starts ~3us earlier.
            dg_sb = wpool.tile([P, 4 * 1152], f8, name="dg_sb", tag="dg")
            x8t = {}
            xtiles = {}
            for t in TILE_ORDER:
                xp = xpool.tile([P, IMG], f8, name=f"x8_{t}", tag=f"x8{t}",
                                bufs=1)
                xtiles[t] = xp
                x8t[t] = xp.rearrange("p (h w) -> p h w", h=PADH)
            r18 = 18 * IMGC
            r34 = 34 * IMGC
            hh8 = (PADH // 2) * IMGC
            t = TILE_ORDER[0]
            nc.scalar.dma_start(dg_sb[:, t * 1152:(t + 1) * 1152],
                                dg_in[:, t * 1152:(t + 1) * 1152])
            nc.sync.dma_start(xtiles[t][:, 0:r18],
                              x8_in[t * P:t * P + P, 0:r18])
            nc.sync.dma_start(xtiles[t][:, r18:r34],
                              x8_in[t * P:t * P + P, r18:r34])
            nc.gpsimd.dma_start(xtiles[t][:, r34:IMG],
                                x8_in[t * P:t * P + P, r34:IMG])
            t = TILE_ORDER[1]
            nc.scalar.dma_start(dg_sb[:, t * 1152:(t + 1) * 1152],
                                dg_in[:, t * 1152:(t + 1) * 1152])
            prm_all = prmpool.tile([P, CT * 11], f32, name="prm_all",
                                   tag="prm")
            nc.gpsimd.dma_start(prm_all[:], prm_in[:])
            ab_sb = prmpool.tile([P, CT], f32, name="ab_sb", tag="ab")
            nc.gpsimd.dma_start(ab_sb[:], ab_in[:])
            nc.sync.dma_start(xtiles[t][:, 0:hh8],
                              x8_in[t * P:t * P + P, 0:hh8])
            nc.sync.dma_start(xtiles[t][:, hh8:IMG],
                              x8_in[t * P:t * P + P, hh8:IMG])
            t = TILE_ORDER[2]
            nc.scalar.dma_start(xtiles[t][:, 0:hh8],
                                x8_in[t * P:t * P + P, 0:hh8])
            nc.scalar.dma_start(xtiles[t][:, hh8:IMG],
                                x8_in[t * P:t * P + P, hh8:IMG])
            nc.gpsimd.dma_start(dg_sb[:, t * 1152:(t + 1) * 1152],
                                dg_in[:, t * 1152:(t + 1) * 1152])
            t = TILE_ORDER[3]
            nc.gpsimd.dma_start(dg_sb[:, t * 1152:(t + 1) * 1152],
                                dg_in[:, t * 1152:(t + 1) * 1152])
            nc.sync.dma_start(xtiles[t][:, 0:hh8],
                              x8_in[t * P:t * P + P, 0:hh8])
            nc.sync.dma_start(xtiles[t][:, hh8:IMG],
                              x8_in[t * P:t * P + P, hh8:IMG])
            wt_sb = wpool.tile([P, 72 * 256], f8, name="wt_sb", tag="wt")
            nc.scalar.dma_start(wt_sb[:, 0:36 * 256], wt_in[:, 0:36 * 256])
            nc.sync.dma_start(wt_sb[:, 36 * 256:72 * 256],
                              wt_in[:, 36 * 256:72 * 256])

            prms = [prm_all[:, t * 11:(t + 1) * 11] for t in range(CT)]

            # the four v images (fp8, 2^9-scaled varying part of mid) in one
            # tile so the dense DoubleRow pair dim can stride between them.
            v4 = vpool.tile([P, CT, PADH, IMGC], f8, name="v4", tag="v4")

            def emit_sums(t, xin):
                """ACT: Square + Identity accumulation passes -> sqs, ms."""
                sqs = smpool.tile([P, 1], f32, name="sqs", tag=f"sqs{t}")
                ms = smpool.tile([P, 1], f32, name="ms", tag=f"ms{t}")
                sscr = ypool.tile([P, HW], f16, name="y", tag="y")
                sscr3 = sscr.rearrange("p (h w) -> p h w", h=H)
                nc.scalar.activation(
                    sscr3, xin,
                    mybir.ActivationFunctionType.Square, accum_out=sqs[:],
                )
                nc.scalar.activation(
                    sscr3, xin,
                    mybir.ActivationFunctionType.Identity, accum_out=ms[:],
                )
                return sqs, ms

            def emit_pre(t, sqs, ms):
                """DVE smalls up to ve (variance + eps); ACT sqrt."""
                mu = smpool.tile([P, 1], f32, name="mu", tag=f"mu{t}")
                nc.vector.tensor_scalar_mul(mu[:], ms[:], 1.0 / HW)
                ex2 = smpool.tile([P, 1], f32, name="ex2", tag="sm")
                nc.vector.tensor_scalar_mul(ex2[:], sqs[:], 1.0 / HW)
                mu2 = smpool.tile([P, 1], f32, name="mu2", tag="sm")
                nc.vector.tensor_mul(mu2[:], mu[:], mu[:])
                ve = smpool.tile([P, 1], f32, name="ve", tag="sm")
                nc.vector.scalar_tensor_tensor(
                    ve[:], mu2[:], -1.0, ex2[:],
                    mybir.AluOpType.mult, mybir.AluOpType.add,
                )
                nc.vector.tensor_scalar_add(ve[:], ve[:], EPS)
                sd = smpool.tile([P, 1], f32, name="sd", tag=f"sd{t}")
                nc.scalar.sqrt(sd[:], ve[:])
                return mu, sd

            def emit_post(t, mu, sd):
                """DVE smalls after sqrt: a, tb (and aw for GpSimd tiles)."""
                prm = prms[t]
                r = smpool.tile([P, 1], f32, name="r", tag="sm")
                nc.vector.reciprocal(r[:], sd[:])
                a = smpool.tile([P, 1], f32, name="a", tag=f"a{t}")
                nc.vector.tensor_mul(a[:], r[:], prm[:, 9:10])
                s9 = smpool.tile([P, 1], f32, name="s9", tag="sm")
                nc.vector.tensor_reduce(
                    s9[:], prm[:, 0:9], mybir.AxisListType.X,
                    mybir.AluOpType.add,
                )
                am = smpool.tile([P, 1], f32, name="am", tag="sm")
                nc.vector.tensor_mul(am[:], a[:], mu[:])
                tb = smpool.tile([P, 1], f32, name="tb", tag=f"tb{t}")
                nc.vector.scalar_tensor_tensor(
                    tb[:], am[:], -1.0, s9[:],
                    mybir.AluOpType.mult, mybir.AluOpType.mult,
                )
                aw = None
                if t in GPS_TILES:
                    aw = smpool.tile([P, 1], f32, name="aw", tag=f"aw{t}")
                    nc.vector.tensor_mul(aw[:], a[:], prm[:, 4:5])
                return a, tb, aw

            def emit_gps_t2(t, aw, tb):
                """GpSimd center-tap pass: t2 = (a*w_c)*x + tb (fp16)."""
                x3 = x8t[t]
                t2t = t2pool.tile([P, HW], f16, name="t2", tag="t2")
                t23 = t2t.rearrange("p (h w) -> p h w", h=H)
                nc.gpsimd.tensor_scalar(
                    t23, x3[:, 1:H + 1, 1:W + 1], aw[:], tb[:],
                    mybir.AluOpType.mult, mybir.AluOpType.add,
                )
                return t2t

            def emit_affine(t, a, tb, t2t, stg):
                """DVE: v4[t] interior = a*staged_acc + (t2 | tb); borders."""
                stg3 = stg.rearrange("p (h w) -> p h w", h=H)
                if t2t is not None:
                    t23 = t2t.rearrange("p (h w) -> p h w", h=H)
                    nc.vector.scalar_tensor_tensor(
                        v4[:, t, 1:H + 1, 1:W + 1], stg3, a[:], t23,
                        mybir.AluOpType.mult, mybir.AluOpType.add,
                    )
                else:
                    nc.vector.tensor_scalar(
                        v4[:, t, 1:H + 1, 1:W + 1], stg3,
                        a[:], tb[:],
                        mybir.AluOpType.mult, mybir.AluOpType.add,
                    )
                _reflect_borders(nc, v4[:, t])

            # ---- depthwise: chunk-outer diagonal-fp8 matmuls; per-chunk
            # psum->fp16 staging copies recycle banks at chunk cadence.
            # Per-tile affine + stats smalls are emitted with ~1-tile lag so
            # they never head-of-line block the staging copies in the DVE
            # FIFO (each waits on ACT stats / the GpSimd t2 pass).
            state = {}   # t -> dict with stats/staging handles
            for j, t in enumerate(TILE_ORDER):
                x3 = x8t[t]
                blocks = DW_BLOCKS[:4] if t in GPS_TILES else DW_BLOCKS
                nblk = len(blocks)
                sqs, ms = emit_sums(t, x3[:, 1:H + 1, 1:W + 1])
                stg = accpool.tile([P, HW], f16, name="acc", tag="dwacc")
                stg3 = stg.rearrange("p (h w) -> p h w", h=H)
                for ch in range(NCHUNK):
                    bank = psum.tile([P, 512], f32, name="bank", tag="bank")
                    for bi, blk in enumerate(blocks):
                        first, last = bi == 0, bi == nblk - 1
                        if len(blk) == 2:
                            (dyA, dxA), (dyB, dxB) = blk
                            lhsT = dg_sb[:, t * 1152 + bi * 256:
                                         t * 1152 + (bi + 1) * 256].rearrange(
                                "p (two m) -> p two m", two=2)
                            stride = (dyB - dyA) * IMGC + (dxB - dxA)
                            base = x3[:, ch * 8 + dyA:ch * 8 + dyA + 8,
                                      dxA:dxA + W]
                            rhs = AP(base.tensor, base.offset,
                                     [list(base.ap[0]), [stride, 2],
                                      [IMGC, 8], [1, W]])
                            nc.tensor.matmul(
                                bank[:], lhsT, rhs,
                                start=first, stop=last, perf_mode=DR)
                        else:
                            (dy, dx), = blk
                            lhsT = dg_sb[:, t * 1152 + 1024:t * 1152 + 1152]
                            rhs = x3[:, ch * 8 + dy:ch * 8 + dy + 8,
                                     dx:dx + W]
                            nc.tensor.matmul(
                                bank[:], lhsT, rhs,
                                start=first, stop=last)
                    nc.vector.tensor_copy(
                        stg3[:, ch * 8:(ch + 1) * 8, :],
                        bank.rearrange("p (h w) -> p h w", h=8),
                    )
                st = {"sqs": sqs, "ms": ms, "stg": stg, "t2": None}
                state[t] = st

                # lagged small-op / affine emission (see module docstring)
                if j == 0:
                    mu, sd = emit_pre(t, sqs, ms)
                    st["a"], st["tb"], _ = emit_post(t, mu, sd)
                elif j == 1:
                    mu, sd = emit_pre(t, sqs, ms)
                    st["a"], st["tb"], _ = emit_post(t, mu, sd)
                    tp = TILE_ORDER[0]
                    emit_affine(tp, state[tp]["a"], state[tp]["tb"],
                                None, state[tp]["stg"])
                elif j == 2:
                    mu, sd = emit_pre(t, sqs, ms)
                    tp = TILE_ORDER[1]
                    emit_affine(tp, state[tp]["a"], state[tp]["tb"],
                                None, state[tp]["stg"])
                    st["a"], st["tb"], aw = emit_post(t, mu, sd)
                    st["t2"] = emit_gps_t2(t, aw, st["tb"])
                else:
                    mu, sd = emit_pre(t, sqs, ms)
                    st["a"], st["tb"], aw = emit_post(t, mu, sd)
                    st["t2"] = emit_gps_t2(t, aw, st["tb"])
                    t0_, t1_ = TILE_ORDER[2], TILE_ORDER[3]
                    emit_affine(t0_, state[t0_]["a"], None,
                                state[t0_]["t2"], state[t0_]["stg"])
                    emit_affine(t1_, state[t1_]["a"], None,
                                state[t1_]["t2"], state[t1_]["stg"])

            # ---- dense 3x3: fp8 DoubleRow, pair dim = two ci tiles.
            # co 0-2 tap-outer (LDW dedup); co 3 chunk-outer so evictions +
            # output DMA overlap its own matmuls (short tail).
            out_rr = (nc.sync, nc.scalar, nc.gpsimd)
            n_out = 0

            def evict(co, gi, bank):
                nonlocal n_out
                o = opool.tile([P, 512], f16, name="o", tag="o")
                if gi % 2 == 0:
                    nc.scalar.activation(
                        o[:], bank[:],
                        mybir.ActivationFunctionType.Identity,
                        bias=ab_sb[:, co:co + 1], scale=OUT_SCALE,
                    )
                else:
                    nc.vector.tensor_scalar(
                        o[:], bank[:], OUT_SCALE, ab_sb[:, co:co + 1],
                        mybir.AluOpType.mult, mybir.AluOpType.add,
                    )
                out_rr[n_out % 3].dma_start(
                    out_ext[co * P:(co + 1) * P, gi * 512:(gi + 1) * 512],
                    o[:],
                )
                n_out += 1

            for co in range(CT - 1):
                groups = [
                    psum.tile([P, 512], f32, name="bank", tag="bank")
                    for _ in range(NCHUNK)
                ]
                for ji, pi in enumerate(PAIR_ORDER):
                    for tap, dy, dx in _taps():
                        idx = (co * 2 + ji) * 9 + tap
                        w_view = wt_sb[:, idx * 256:(idx + 1) * 256].rearrange(
                            "p (two m) -> p two m", two=2)
                        for ch in range(NCHUNK):
                            rhs = v4[:, 2 * pi:2 * pi + 2,
                                     ch * 8 + dy:ch * 8 + dy + 8, dx:dx + W]
                            nc.tensor.matmul(
                                groups[ch][:], w_view, rhs,
                                start=(ji == 0 and tap == 0),
                                stop=(ji == 1 and tap == 8),
                                perf_mode=DR,
                            )
                for gi in range(NCHUNK):
                    evict(co, gi, groups[gi])

            co = CT - 1
            for gi in range(NCHUNK):
                bank = psum.tile([P, 512], f32, name="bank", tag="bank")
                n_acc = 0
                for ji, pi in enumerate(PAIR_ORDER):
                    for tap, dy, dx in _taps():
                        idx = (co * 2 + ji) * 9 + tap
                        w_view = wt_sb[:, idx * 256:(idx + 1) * 256].rearrange(
                            "p (two m) -> p two m", two=2)
                        rhs = v4[:, 2 * pi:2 * pi + 2,
                                 gi * 8 + dy:gi * 8 + dy + 8, dx:dx + W]
                        nc.tensor.matmul(
                            bank[:], w_view, rhs,
                            start=(n_acc == 0), stop=(n_acc == 17),
                            perf_mode=DR,
                        )
                        n_acc += 1
                evict(co, gi, bank)

    nc.compile()
    _dedup_ldweights(nc)
    return nc


def kernel(x, w_spatial, w_pointwise, bias, conv_w, conv_b):
    global LAST_EXEC_NS
    if "nc" not in _CACHE:
        _CACHE["nc"] = _build()
    nc = _CACHE["nc"]

    xf = np.asarray(x, dtype=np.float32).astype(np.float16)
    xpad = np.pad(xf, ((0, 0), (0, 0), (1, 1), (1, 1)), mode="reflect")
    ws = np.asarray(w_spatial, dtype=np.float32).reshape(B, C, 9)
    wp = np.asarray(w_pointwise, dtype=np.float32).reshape(B, C)
    bi = np.asarray(bias, dtype=np.float32).reshape(B, C)
    cw = np.asarray(conv_w, dtype=np.float32)
    cb = np.asarray(conv_b, dtype=np.float32)

    # shared dense weights, fp8, emission-order blocks [p, ko, m]:
    # wt[p, ((co*2+j)*9+tap)*256 + ko*128 + m]
    #   = fp8(conv_w[co*128+m, ci*128+p, tap] * 2^M_SC), ci = CI_PAIRS[pi][ko]
    w8 = (cw.reshape(C, C, 9) * (2.0 ** M_SC)).astype(E4)
    wt = np.zeros((P, 72 * 256), dtype=E4)
    w8v = w8.view(np.uint8)
    wtv = wt.view(np.uint8)
    for co in range(CT):
        for ji, pi in enumerate(PAIR_ORDER):
            for tap in range(9):
                idx = (co * 2 + ji) * 9 + tap
                for ko in range(2):
                    ci_t = CI_PAIRS[pi][ko]
                    blk = w8v[co * P:(co + 1) * P,
                              ci_t * P:(ci_t + 1) * P, tap].T
                    wtv[:, idx * 256 + ko * P:idx * 256 + (ko + 1) * P] = blk

    # dense bias constant, exact on host: AB[co] = S@b + conv_b
    S = cw.sum(axis=(2, 3)).astype(np.float64)        # [co, ci]

    in_maps = []
    for b in range(B):
        # fp8 padded x, rows stored at stride 72
        x8 = np.zeros((C, PADH, IMGC), dtype=E4)
        x8[:, :, 0:PADW] = xpad[b].astype(E4)
        # per-channel params; the fp8-rounded taps (scaled 2^S_SC) go in so
        # the mu*s9 correction matches the diag weights exactly
        wsq8 = (ws[b] * (2.0 ** S_SC)).astype(E4)
        prm = np.zeros((CT, P, 11), dtype=np.float32)
        prm[:, :, 0:9] = wsq8.astype(np.float32).reshape(CT, P, 9)
        prm[:, :, 9] = wp[b].reshape(CT, P)
        prm = np.ascontiguousarray(prm.transpose(1, 0, 2).reshape(P, CT * 11))

        ab = (S @ bi[b].astype(np.float64) + cb).astype(np.float32)
        ab = np.ascontiguousarray(ab.reshape(CT, P).T)   # [p, CT]

        # diagonal fp8 depthwise weights, one 1152-col block per tile
        dg = np.zeros((P, 4 * 1152), dtype=E4)
        idxp = np.arange(P)
        for t in range(CT):
            wsq = wsq8[t * P:(t + 1) * P]
            for bi2, blk in enumerate(DW_BLOCKS):
                if len(blk) == 2:
                    for ko, (dy, dx) in enumerate(blk):
                        dg[idxp, t * 1152 + bi2 * 256 + ko * P + idxp] = \
                            wsq[:, dy * 3 + dx]
                else:
                    (dy, dx), = blk
                    dg[idxp, t * 1152 + 1024 + idxp] = wsq[:, dy * 3 + dx]

        in_maps.append({
            "x8": x8.reshape(C, IMG),
            "wt": wt,
            "dg": dg,
            "prm": prm,
            "ab": ab,
        })

    res = run_bass_kernel_spmd(
        nc, in_maps, list(range(N_CORES)), trace=_TRACE
    )
    LAST_EXEC_NS = res.exec_time_ns
    out = np.stack([
        res.results[b]["out"].astype(np.float32).reshape(C, H, W)
        for b in range(B)
    ])
    return out


# revision 7
# speedup vs baseline: 1.0304x; 1.0304x over previous
"""Trainium2 Bass kernel for AdaConv2d (instance-norm + per-sample dynamic
depthwise 3x3 conv + per-channel scale/bias + shared dense 3x3 conv, reflect
padding everywhere).

Data-parallel over batch: 8 samples -> 8 NeuronCores, one sample per core.

Math (per sample, per channel c):
    xn   = (x - mu_c) * rsqrt(var_c + eps)
    mid  = wp_c * depthwise3x3(reflect_pad(xn); ws_c) + b_c
         = v_c + b_c    with v = a*dw(x) - a*mu*s9,  a = wp*rsqrt(var+eps)
    out  = dense3x3(reflect_pad(mid); conv_w) + conv_b
         = dense3x3(reflect_pad(v); conv_w) + S@b + conv_b
      where S[co,ci] = sum_taps conv_w  (reflect padding makes the per-channel
      constant b contribute exactly S@b at every output pixel).  S@b + conv_b
      is exact host-side preprocessing of the weights/bias (input `ab`).

The v split is what makes fp8 viable: v has std ~0.0075 (vs mid's ~0.05
dominated by the per-channel constant b), so quantizing v*2^9 to fp8e4m3
keeps the dense-conv error ~1e-2 relative (gate is 2e-2).  The dense conv
runs as fp8 DoubleRow matmuls: each matmul contracts 256 = 2 channel tiles
(the pair dim of the moving AP strides between two v images that live in
one SBUF tile).  On HW the PE streams 1 moving column/cycle regardless of
mode, so DR's win is halving the accumulation passes (18 per co tile).

Depthwise runs on the PE as diagonal-fp8-weight matmuls, DoubleRow-pairing
two taps per matmul (5 blocks per tile: 4 pairs + center), chunk-outer so
each 8-row chunk finishes all its tap blocks back to back and its
psum->fp16 staging copy (DVE) recycles the bank at chunk cadence -- the
staging copies are the ONLY bulk DVE work, so banks never wait.  Stats are
spread so no engine FIFO head-of-line blocks another: sums on ACT
(Square/Identity with accum) + sqrt; pre-sqrt smalls on GpSimd; post-sqrt
smalls on DVE.  The per-tile affine v = a*acc + tb runs on the otherwise
idle GpSimd engine (tensor_scalar is its native form) with ~1-tile lag,
followed by the reflect borders on the same engine.  DVE was the
oversubscribed engine in earlier variants (copies + affines + smalls >
the PE's 8.7us/tile dw rate); GpSimd affines are what fixed that.

Head: the first matmul's dependencies (tile-2 diag weights + x rows 0-17)
ride the otherwise-idle tensor-engine DMA ring, with the rest of tile 2
split across the sync/gpsimd rings, so the PE starts ~3us earlier.  Tail:
the last dense co tile runs chunk-outer (18 accum MMs per chunk, then
immediate eviction + output DMA) so only one chunk's eviction remains
after the final matmul.
"""

import os
import sys
import types

import numpy as np
import ml_dtypes

B, C, H, W = 8, 512, 64, 64
KS = 3
EPS = 1e-5
N_CORES = 8
P = 128
CT = C // P            # 4 channel tiles
PADH = H + 2           # 66
PADW = W + 2           # 66
IMGC = 72              # stored row stride (16B-aligned images: 66*72 = 4752)
IMG = PADH * IMGC      # 4752
HW = H * W             # 4096
NCHUNK = 8             # 8-row output chunks -> one psum bank each
TILE_ORDER = (2, 3, 0, 1)  # depthwise production order (dense eats 2,3 first)
CI_PAIRS = ((0, 1), (2, 3))
PAIR_ORDER = (1, 0)    # dense contraction: pair (2,3) first (ready early)
K_SC = 9               # v scale 2^9
M_SC = 9               # dense weight scale 2^9
S_SC = 9               # depthwise diag weight scale 2^9
OUT_SCALE = float(2.0 ** (-(K_SC + M_SC)))
# depthwise tap blocks: 4 DoubleRow pairs + 1 single (center)
DW_BLOCKS = (((0, 0), (0, 2)), ((1, 0), (1, 2)), ((2, 0), (2, 2)),
             ((0, 1), (2, 1)), ((1, 1),))

E4 = ml_dtypes.float8_e4m3


def _install_ntff_hook():
    """Register the NTFF profiling hook that concourse expects under axon
    (missing antenv.axon_hooks module in this image)."""
    if "antenv.axon_hooks" in sys.modules:
        return
    try:
        mod = types.ModuleType("antenv.axon_hooks")
        holder = [None]
        mod.set_axon_ntff_profile_hook = lambda h: holder.__setitem__(0, h)
        mod.get_axon_ntff_profile_hook = lambda: holder[0]
        sys.modules["antenv.axon_hooks"] = mod
        from trn_agent_boot.trn_boot import _ntff_profile_via_ctypes

        hook = _ntff_profile_via_ctypes("/opt/axon/libaxon_pjrt.so")
        mod.set_axon_ntff_profile_hook(hook)
    except Exception:
        sys.modules.pop("antenv.axon_hooks", None)


_TRACE = os.environ.get("BASS_KERNEL_TRACE") == "1"
if _TRACE:
    _install_ntff_hook()

import concourse.tile as tile
from concourse import bacc, mybir
import concourse.bass_utils as bass_utils
from concourse.bass_utils import run_bass_kernel_spmd
from concourse.ap import AP

if _TRACE:
    bass_utils.upload_artifacts = lambda d: d

LAST_EXEC_NS = None
_CACHE = {}


def _taps():
    for tap in range(KS * KS):
        yield tap, tap // KS, tap % KS


def _reflect_borders(nc, img3):
    """Fill the 1-wide reflect border of a [128, 66, 72] image whose
    interior [1:65, 1:65] is already populated (cols first, then full rows
    so the corners come out as reflect-of-reflect, matching np.pad)."""
    nc.vector.tensor_copy(img3[:, 1:H + 1, 0:1], img3[:, 1:H + 1, 2:3])
    nc.vector.tensor_copy(img3[:, 1:H + 1, PADW - 1:PADW],
                          img3[:, 1:H + 1, PADW - 3:PADW - 2])
    nc.vector.tensor_copy(img3[:, 0:1, 0:PADW], img3[:, 2:3, 0:PADW])
    nc.vector.tensor_copy(img3[:, PADH - 1:PADH, 0:PADW],
                          img3[:, PADH - 3:PADH - 2, 0:PADW])


def _dedup_ldweights(nc):
    """Drop InstLdweights whose weights AP is identical to the previous
    weight load on the PE stream (bacc splits every matmul into LDW+MM;
    with one weight block reused across 8 chunk matmuls, 7 of 8 loads are
    redundant and serialize with the matmuls).  LDWs carrying semaphore
    waits/updates are kept."""
    n_removed = 0
    for f in nc.m.functions:
        for bb in f.blocks:
            insts = bb.instructions
            keep = []
            last_key = None
            for inst in insts:
                tn = type(inst).__name__
                if tn == "InstLdweights":
                    si = inst.sync_info
                    has_sync = si is not None and (
                        len(si.on_wait) > 0 or len(si.on_update) > 0
                    )
                    key = repr(inst.ins[0])
                    if key == last_key and not has_sync:
                        n_removed += 1
                        continue
                    last_key = key
                elif tn == "InstMatmult":
                    if getattr(inst, "is_transpose", False):
                        last_key = None
                keep.append(inst)
            if len(keep) != len(insts):
                bb.instructions = keep
    return n_removed


def _build():
    nc = bacc.Bacc("TRN2", target_bir_lowering=False, debug=False,
                   num_devices=N_CORES)
    f32 = mybir.dt.float32
    f16 = mybir.dt.float16
    f8 = mybir.dt.float8e4
    DR = mybir.MatmulPerfMode.DoubleRow

    x8_in = nc.dram_tensor("x8", [C, IMG], f8, kind="ExternalInput").ap()
    wt_in = nc.dram_tensor("wt", [P, 72 * 256], f8, kind="ExternalInput").ap()
    dg_in = nc.dram_tensor("dg", [P, 4 * 1152], f8, kind="ExternalInput").ap()
    prm_in = nc.dram_tensor("prm", [P, CT * 11], f32,
                            kind="ExternalInput").ap()
    ab_in = nc.dram_tensor("ab", [P, CT], f32, kind="ExternalInput").ap()
    out_ext = nc.dram_tensor("out", [C, HW], f16, kind="ExternalOutput").ap()

    with tile.TileContext(nc) as tc:
        with (
            tc.tile_pool(name="wpool", bufs=1) as wpool,
            tc.tile_pool(name="xpool", bufs=2) as xpool,
            tc.tile_pool(name="vpool", bufs=1) as vpool,
            tc.tile_pool(name="accpool", bufs=2) as accpool,
            tc.tile_pool(name="ypool", bufs=2) as ypool,
            tc.tile_pool(name="smpool", bufs=8) as smpool,
            tc.tile_pool(name="prmpool", bufs=4) as prmpool,
            tc.tile_pool(name="opool", bufs=8) as opool,
            tc.tile_pool(name="psum", bufs=8, space="PSUM") as psum,
        ):
            # ---- input DMAs: the first matmul (tile 2 chunk 0) needs only
            # the tile-2 diag weights + x rows 0-17; both ride the idle
            # tensor-engine ring so the PE starts ~3us earlier.
            dg_sb = wpool.tile([P, 4 * 1152], f8, name="dg_sb", tag="dg")
            x8t = {}
            xtiles = {}
            for t in TILE_ORDER:
                xp = xpool.tile([P, IMG], f8, name=f"x8_{t}", tag=f"x8{t}",
                                bufs=1)
                xtiles[t] = xp
                x8t[t] = xp.rearrange("p (h w) -> p h w", h=PADH)
            r18 = 18 * IMGC
            r34 = 34 * IMGC
            hh8 = (PADH // 2) * IMGC
            t = TILE_ORDER[0]
            nc.scalar.dma_start(dg_sb[:, t * 1152:(t + 1) * 1152],
                                dg_in[:, t * 1152:(t + 1) * 1152])
            nc.sync.dma_start(xtiles[t][:, 0:r18],
                              x8_in[t * P:t * P + P, 0:r18])
            nc.sync.dma_start(xtiles[t][:, r18:r34],
                              x8_in[t * P:t * P + P, r18:r34])
            nc.gpsimd.dma_start(xtiles[t][:, r34:IMG],
                                x8_in[t * P:t * P + P, r34:IMG])
            t = TILE_ORDER[1]
            nc.scalar.dma_start(dg_sb[:, t * 1152:(t + 1) * 1152],
                                dg_in[:, t * 1152:(t + 1) * 1152])
            prm_all = prmpool.tile([P, CT * 11], f32, name="prm_all",
                                   tag="prm")
            nc.gpsimd.dma_start(prm_all[:], prm_in[:])
            ab_sb = prmpool.tile([P, CT], f32, name="ab_sb", tag="ab")
            nc.gpsimd.dma_start(ab_sb[:], ab_in[:])
            nc.sync.dma_start(xtiles[t][:, 0:hh8],
                              x8_in[t * P:t * P + P, 0:hh8])
            nc.sync.dma_start(xtiles[t][:, hh8:IMG],
                              x8_in[t * P:t * P + P, hh8:IMG])
            t = TILE_ORDER[2]
            nc.scalar.dma_start(xtiles[t][:, 0:hh8],
                                x8_in[t * P:t * P + P, 0:hh8])
            nc.scalar.dma_start(xtiles[t][:, hh8:IMG],
                                x8_in[t * P:t * P + P, hh8:IMG])
            nc.gpsimd.dma_start(dg_sb[:, t * 1152:(t + 1) * 1152],
                                dg_in[:, t * 1152:(t + 1) * 1152])
            t = TILE_ORDER[3]
            nc.gpsimd.dma_start(dg_sb[:, t * 1152:(t + 1) * 1152],
                                dg_in[:, t * 1152:(t + 1) * 1152])
            nc.sync.dma_start(xtiles[t][:, 0:hh8],
                              x8_in[t * P:t * P + P, 0:hh8])
            nc.sync.dma_start(xtiles[t][:, hh8:IMG],
                              x8_in[t * P:t * P + P, hh8:IMG])
            wt_sb = wpool.tile([P, 72 * 256], f8, name="wt_sb", tag="wt")
            nc.scalar.dma_start(wt_sb[:, 0:36 * 256], wt_in[:, 0:36 * 256])
            nc.sync.dma_start(wt_sb[:, 36 * 256:72 * 256],
                              wt_in[:, 36 * 256:72 * 256])

            prms = [prm_all[:, t * 11:(t + 1) * 11] for t in range(CT)]

            # the four v images (fp8, 2^9-scaled varying part of mid) in one
            # tile so the dense DoubleRow pair dim can stride between them.
            v4 = vpool.tile([P, CT, PADH, IMGC], f8, name="v4", tag="v4")

            def emit_sums(t, xin):
                """ACT: Square + Identity accumulation passes -> sqs, ms."""
                sqs = smpool.tile([P, 1], f32, name="sqs", tag=f"sqs{t}")
                ms = smpool.tile([P, 1], f32, name="ms", tag=f"ms{t}")
                sscr = ypool.tile([P, HW], f16, name="y", tag="y")
                sscr3 = sscr.rearrange("p (h w) -> p h w", h=H)
                nc.scalar.activation(
                    sscr3, xin,
                    mybir.ActivationFunctionType.Square, accum_out=sqs[:],
                )
                nc.scalar.activation(
                    sscr3, xin,
                    mybir.ActivationFunctionType.Identity, accum_out=ms[:],
                )
                return sqs, ms

            def emit_pre(t, sqs, ms):
                """GpSimd smalls up to ve (variance + eps); ACT sqrt.
                Keeping these off the DVE FIFO is what lets the staging
                copies flow at chunk cadence."""
                mu = smpool.tile([P, 1], f32, name="mu", tag=f"mu{t}")
                nc.gpsimd.tensor_scalar_mul(mu[:], ms[:], 1.0 / HW)
                ex2 = smpool.tile([P, 1], f32, name="ex2", tag="sm")
                nc.gpsimd.tensor_scalar_mul(ex2[:], sqs[:], 1.0 / HW)
                mu2 = smpool.tile([P, 1], f32, name="mu2", tag="sm")
                nc.gpsimd.tensor_mul(mu2[:], mu[:], mu[:])
                ve = smpool.tile([P, 1], f32, name="ve", tag="sm")
                nc.gpsimd.tensor_sub(ve[:], ex2[:], mu2[:])
                nc.gpsimd.tensor_scalar_add(ve[:], ve[:], EPS)
                sd = smpool.tile([P, 1], f32, name="sd", tag=f"sd{t}")
                nc.scalar.sqrt(sd[:], ve[:])
                return mu, sd

            def emit_post(t, mu, sd):
                """DVE smalls after sqrt: a (affine scale), tb (offset)."""
                prm = prms[t]
                r = smpool.tile([P, 1], f32, name="r", tag="sm")
                nc.vector.reciprocal(r[:], sd[:])
                a = smpool.tile([P, 1], f32, name="a", tag=f"a{t}")
                nc.vector.tensor_mul(a[:], r[:], prm[:, 9:10])
                s9 = smpool.tile([P, 1], f32, name="s9", tag="sm")
                nc.vector.tensor_reduce(
                    s9[:], prm[:, 0:9], mybir.AxisListType.X,
                    mybir.AluOpType.add,
                )
                am = smpool.tile([P, 1], f32, name="am", tag="sm")
                nc.vector.tensor_mul(am[:], a[:], mu[:])
                tb = smpool.tile([P, 1], f32, name="tb", tag=f"tb{t}")
                nc.vector.scalar_tensor_tensor(
                    tb[:], am[:], -1.0, s9[:],
                    mybir.AluOpType.mult, mybir.AluOpType.mult,
                )
                return a, tb

            def emit_affine(t, a, tb, stg):
                """GpSimd: v4[t] interior = a*staged_acc + tb, then the
                reflect borders (same engine, so no cross-engine bubble)."""
                stg3 = stg.rearrange("p (h w) -> p h w", h=H)
                nc.gpsimd.tensor_scalar(
                    v4[:, t, 1:H + 1, 1:W + 1], stg3,
                    a[:], tb[:],
                    mybir.AluOpType.mult, mybir.AluOpType.add,
                )
                img3 = v4[:, t]
                nc.gpsimd.tensor_copy(img3[:, 1:H + 1, 0:1],
                                      img3[:, 1:H + 1, 2:3])
                nc.gpsimd.tensor_copy(img3[:, 1:H + 1, PADW - 1:PADW],
                                      img3[:, 1:H + 1, PADW - 3:PADW - 2])
                nc.gpsimd.tensor_copy(img3[:, 0:1, 0:PADW],
                                      img3[:, 2:3, 0:PADW])
                nc.gpsimd.tensor_copy(img3[:, PADH - 1:PADH, 0:PADW],
                                      img3[:, PADH - 3:PADH - 2, 0:PADW])

            # ---- depthwise: chunk-outer diagonal-fp8 matmuls; per-chunk
            # psum->fp16 staging copies (the ONLY bulk DVE work) recycle the
            # 8 psum banks at chunk cadence.  Stats ride ACT (sums, sqrt),
            # GpSimd (pre-sqrt smalls) and DVE (post-sqrt smalls); the
            # per-tile affine runs on GpSimd with ~1-tile lag.
            state = {}   # t -> dict with stats/staging handles
            for j, t in enumerate(TILE_ORDER):
                x3 = x8t[t]
                sqs, ms = emit_sums(t, x3[:, 1:H + 1, 1:W + 1])
                if j >= 1:
                    tp = TILE_ORDER[j - 1]
                    emit_affine(tp, state[tp]["a"], state[tp]["tb"],
                                state[tp]["stg"])
                mu, sd = emit_pre(t, sqs, ms)
                stg = accpool.tile([P, HW], f16, name="acc", tag="dwacc")
                stg3 = stg.rearrange("p (h w) -> p h w", h=H)
                for ch in range(NCHUNK):
                    bank = psum.tile([P, 512], f32, name="bank", tag="bank")
                    for bi, blk in enumerate(DW_BLOCKS):
                        first, last = bi == 0, bi == len(DW_BLOCKS) - 1
                        if len(blk) == 2:
                            (dyA, dxA), (dyB, dxB) = blk
                            lhsT = dg_sb[:, t * 1152 + bi * 256:
                                         t * 1152 + (bi + 1) * 256].rearrange(
                                "p (two m) -> p two m", two=2)
                            stride = (dyB - dyA) * IMGC + (dxB - dxA)
                            base = x3[:, ch * 8 + dyA:ch * 8 + dyA + 8,
                                      dxA:dxA + W]
                            rhs = AP(base.tensor, base.offset,
                                     [list(base.ap[0]), [stride, 2],
                                      [IMGC, 8], [1, W]])
                            nc.tensor.matmul(
                                bank[:], lhsT, rhs,
                                start=first, stop=last, perf_mode=DR)
                        else:
                            (dy, dx), = blk
                            lhsT = dg_sb[:, t * 1152 + 1024:t * 1152 + 1152]
                            rhs = x3[:, ch * 8 + dy:ch * 8 + dy + 8,
                                     dx:dx + W]
                            nc.tensor.matmul(
                                bank[:], lhsT, rhs,
                                start=first, stop=last)
                    nc.vector.tensor_copy(
                        stg3[:, ch * 8:(ch + 1) * 8, :],
                        bank.rearrange("p (h w) -> p h w", h=8),
                    )
                a, tb = emit_post(t, mu, sd)
                state[t] = {"a": a, "tb": tb, "stg": stg}
            t_last = TILE_ORDER[-1]
            emit_affine(t_last, state[t_last]["a"], state[t_last]["tb"],
                        state[t_last]["stg"])

            # ---- dense 3x3: fp8 DoubleRow, pair dim = two ci tiles.
            # co 0-2 tap-outer (LDW dedup); co 3 chunk-outer so evictions +
            # output DMA overlap its own matmuls (short tail).
            out_rr = (nc.sync, nc.scalar, nc.gpsimd)
            n_out = 0

            def evict(co, gi, bank):
                nonlocal n_out
                o = opool.tile([P, 512], f16, name="o", tag="o")
                if gi % 2 == 0:
                    nc.scalar.activation(
                        o[:], bank[:],
                        mybir.ActivationFunctionType.Identity,
                        bias=ab_sb[:, co:co + 1], scale=OUT_SCALE,
                    )
                else:
                    nc.vector.tensor_scalar(
                        o[:], bank[:], OUT_SCALE, ab_sb[:, co:co + 1],
                        mybir.AluOpType.mult, mybir.AluOpType.add,
                    )
                out_rr[n_out % 3].dma_start(
                    out_ext[co * P:(co + 1) * P, gi * 512:(gi + 1) * 512],
                    o[:],
                )
                n_out += 1

            for co in range(CT - 1):
                groups = [
                    psum.tile([P, 512], f32, name="bank", tag="bank")
                    for _ in range(NCHUNK)
                ]
                for ji, pi in enumerate(PAIR_ORDER):
                    for tap, dy, dx in _taps():
                        idx = (co * 2 + ji) * 9 + tap
                        w_view = wt_sb[:, idx * 256:(idx + 1) * 256].rearrange(
                            "p (two m) -> p two m", two=2)
                        for ch in range(NCHUNK):
                            rhs = v4[:, 2 * pi:2 * pi + 2,
                                     ch * 8 + dy:ch * 8 + dy + 8, dx:dx + W]
                            nc.tensor.matmul(
                                groups[ch][:], w_view, rhs,
                                start=(ji == 0 and tap == 0),
                                stop=(ji == 1 and tap == 8),
                                perf_mode=DR,
                            )
                for gi in range(NCHUNK):
                    evict(co, gi, groups[gi])

            co = CT - 1
            for gi in range(NCHUNK):
                bank = psum.tile([P, 512], f32, name="bank", tag="bank")
                n_acc = 0
                for ji, pi in enumerate(PAIR_ORDER):
                    for tap, dy, dx in _taps():
                        idx = (co * 2 + ji) * 9 + tap
                        w_view = wt_sb[:, idx * 256:(idx + 1) * 256].rearrange(
                            "p (two m) -> p two m", two=2)
                        rhs = v4[:, 2 * pi:2 * pi + 2,
                                 gi * 8 + dy:gi * 8 + dy + 8, dx:dx + W]
                        nc.tensor.matmul(
                            bank[:], w_view, rhs,
                            start=(n_acc == 0), stop=(n_acc == 17),
                            perf_mode=DR,
                        )
                        n_acc += 1
                evict(co, gi, bank)

    nc.compile()
    _dedup_ldweights(nc)
    return nc


def kernel(x, w_spatial, w_pointwise, bias, conv_w, conv_b):
    global LAST_EXEC_NS
    if "nc" not in _CACHE:
        _CACHE["nc"] = _build()
    nc = _CACHE["nc"]

    xf = np.asarray(x, dtype=np.float32).astype(np.float16)
    xpad = np.pad(xf, ((0, 0), (0, 0), (1, 1), (1, 1)), mode="reflect")
    ws = np.asarray(w_spatial, dtype=np.float32).reshape(B, C, 9)
    wp = np.asarray(w_pointwise, dtype=np.float32).reshape(B, C)
    bi = np.asarray(bias, dtype=np.float32).reshape(B, C)
    cw = np.asarray(conv_w, dtype=np.float32)
    cb = np.asarray(conv_b, dtype=np.float32)

    # shared dense weights, fp8, emission-order blocks [p, ko, m]:
    # wt[p, ((co*2+j)*9+tap)*256 + ko*128 + m]
    #   = fp8(conv_w[co*128+m, ci*128+p, tap] * 2^M_SC), ci = CI_PAIRS[pi][ko]
    w8 = (cw.reshape(C, C, 9) * (2.0 ** M_SC)).astype(E4)
    wt = np.zeros((P, 72 * 256), dtype=E4)
    w8v = w8.view(np.uint8)
    wtv = wt.view(np.uint8)
    for co in range(CT):
        for ji, pi in enumerate(PAIR_ORDER):
            for tap in range(9):
                idx = (co * 2 + ji) * 9 + tap
                for ko in range(2):
                    ci_t = CI_PAIRS[pi][ko]
                    blk = w8v[co * P:(co + 1) * P,
                              ci_t * P:(ci_t + 1) * P, tap].T
                    wtv[:, idx * 256 + ko * P:idx * 256 + (ko + 1) * P] = blk

    # dense bias constant, exact on host: AB[co] = S@b + conv_b
    S = cw.sum(axis=(2, 3)).astype(np.float64)        # [co, ci]

    in_maps = []
    for b in range(B):
        # fp8 padded x, rows stored at stride 72
        x8 = np.zeros((C, PADH, IMGC), dtype=E4)
        x8[:, :, 0:PADW] = xpad[b].astype(E4)
        # per-channel params; the fp8-rounded taps (scaled 2^S_SC) go in so
        # the mu*s9 correction matches the diag weights exactly
        wsq8 = (ws[b] * (2.0 ** S_SC)).astype(E4)
        prm = np.zeros((CT, P, 11), dtype=np.float32)
        prm[:, :, 0:9] = wsq8.astype(np.float32).reshape(CT, P, 9)
        prm[:, :, 9] = wp[b].reshape(CT, P)
        prm = np.ascontiguousarray(prm.transpose(1, 0, 2).reshape(P, CT * 11))

        ab = (S @ bi[b].astype(np.float64) + cb).astype(np.float32)
        ab = np.ascontiguousarray(ab.reshape(CT, P).T)   # [p, CT]

        # diagonal fp8 depthwise weights, one 1152-col block per tile
        dg = np.zeros((P, 4 * 1152), dtype=E4)
        idxp = np.arange(P)
        for t in range(CT):
            wsq = wsq8[t * P:(t + 1) * P]
            for bi2, blk in enumerate(DW_BLOCKS):
                if len(blk) == 2:
                    for ko, (dy, dx) in enumerate(blk):
                        dg[idxp, t * 1152 + bi2 * 256 + ko * P + idxp] = \
                            wsq[:, dy * 3 + dx]
                else:
                    (dy, dx), = blk
                    dg[idxp, t * 1152 + 1024 + idxp] = wsq[:, dy * 3 + dx]

        in_maps.append({
            "x8": x8.reshape(C, IMG),
            "wt": wt,
            "dg": dg,
            "prm": prm,
            "ab": ab,
        })

    res = run_bass_kernel_spmd(
        nc, in_maps, list(range(N_CORES)), trace=_TRACE
    )
    LAST_EXEC_NS = res.exec_time_ns
    out = np.stack([
        res.results[b]["out"].astype(np.float32).reshape(C, H, W)
        for b in range(B)
    ])
    return out
